# revision 46
# baseline (speedup 1.0000x reference)
"""Trainium2 Bass kernel for MultiHeadLatentAttention.

Problem (hardcoded): B=2, S=2048, DIN=2048, DOUT=2048, LATENT=512,
HEADS=16, head_dim=128, fp32 in/out, causal attention, softmax scale
1/sqrt(S).

Sharding: 8 cores = batch (2) x head-groups (4 groups of 4 heads).
Each core computes, for its (batch b, head group g):
    q = x_b @ Wq[:, g]            (as q^T, head-dim on partitions, RoPE'd)
    kv_lat = x_b @ Wl             (as kv_lat^T)
    k^T = Wu_k[:, g]^T @ kv_lat^T (RoPE'd), v = kv_lat @ Wu_v[:, g]
    per head: E^T = exp(scale * K Q^T) (causal), O^T = V^T E^T / R
    pout = O @ Wp[g rows, :]      (partial over head-group dims)
Host sums the 4 per-group partials for each batch (fp16 partials).

All matmul operands are fp16 (PSUM accumulation is fp32): fp16 streams
at 1 cycle/moving-element at any width (no fp32r 256-wide floor), makes
the stationary-operand loads FWL-eligible, and halves DMA + SBUF + DVE
traffic. Measured end-to-end error vs the fp64 oracle is ~4e-4.
"""

import math
import os

import numpy as np

import concourse.bass as bass
import concourse.mybir as mybir
import concourse.tile as tile
from concourse import bacc, bass_utils

# ---- problem constants (self-contained; do not read spec/reference) ----
B = 2
S = 2048
DIN = 2048
DOUT = 2048
LATENT = 512
HEADS = 16
HD = 128                 # head dim
NCORES = 8
GROUPS = 4               # head groups (tensor parallel dimension)
GH = HEADS // GROUPS     # heads per group = 4
GD = GH * HD             # dims per group = 512

SB = 512                 # s-block width for projection stages
NSB = S // SB            # 4
KT = DIN // 128          # 16 contraction tiles over DIN
LT = LATENT // 128       # 4 contraction tiles over LATENT
QC = 512                 # q-chunk width in attention
NQC = S // QC            # 4
NKB = S // 128           # 16 k-blocks

F32 = mybir.dt.float32
F16 = mybir.dt.float16
SCALE = 1.0 / math.sqrt(float(S))


def build_nc(stage=None, repeat=None):
    if stage is None:
        stage = int(os.environ.get("K_STAGE", "4"))
    if repeat is None:
        repeat = int(os.environ.get("K_REPEAT", "1"))
    nc = bacc.Bacc(
        "TRN2", target_bir_lowering=False, debug=False, num_devices=NCORES
    )
    _build_body(nc, stage, repeat)
    nc.compile()
    return nc


def _build_body(nc, stage, repeat=1):
    xT = nc.dram_tensor("xT", [DIN, S], F16, kind="ExternalInput")
    wq = nc.dram_tensor("wq", [DIN, GD], F16, kind="ExternalInput")
    wl = nc.dram_tensor("wl", [DIN, LATENT], F16, kind="ExternalInput")
    wuk = nc.dram_tensor("wuk", [LATENT, GD], F16, kind="ExternalInput")
    wuv = nc.dram_tensor("wuv", [LATENT, GD], F16, kind="ExternalInput")
    wp = nc.dram_tensor("wp", [GD, DOUT], F16, kind="ExternalInput")
    cosT = nc.dram_tensor("cosT", [HD, S], F32, kind="ExternalInput")
    sinT = nc.dram_tensor("sinT", [HD, S], F32, kind="ExternalInput")
    masksd = nc.dram_tensor("masks", [128, 128], F16, kind="ExternalInput")
    eyed = nc.dram_tensor("eye", [128, 128], F16, kind="ExternalInput")
    pout = nc.dram_tensor("pout", [S, DOUT], F16, kind="ExternalOutput")

    xT_t = xT.rearrange("(ko ki) s -> ki ko s", ki=128)       # [128,16,S]
    wq_t = wq.rearrange("(ko ki) d -> ki ko d", ki=128)       # [128,16,GD]
    wl_t = wl.rearrange("(ko ki) l -> ki ko l", ki=128)       # [128,16,LAT]
    wuk_t = wuk.rearrange("(lo li) d -> li lo d", li=128)     # [128,4,GD]
    wuv_t = wuv.rearrange("(lo li) d -> li lo d", li=128)     # [128,4,GD]
    wp_t = wp.rearrange("(dt di) e -> di dt e", di=128)       # [128,4,DOUT]

    with tile.TileContext(nc) as tc:
      for _rep in range(repeat):
        with (
            tc.tile_pool(name="persist", bufs=1) as persist,
            tc.tile_pool(name="kvres", bufs=1) as kvres,
        ):
            # manually-released pools (right-side stack, LIFO):
            # qres outlives xt/cs
            qres = tc.alloc_tile_pool(name="qres", bufs=1, side="right")
            xtp = tc.alloc_tile_pool(name="xt", bufs=3, side="right")
            cs = tc.alloc_tile_pool(name="cs", bufs=1, side="right")
            cos_sb = cs.tile([HD, S], F32)
            sin_sb = cs.tile([HD, S], F32)
            eye_sb = persist.tile([128, 128], F16)
            masks_sb = persist.tile([128, 128], F16)

            # PE warm-up: ~4us of junk matmuls while the first DMAs land,
            # so the HAM clock-gate reaches 2.4 GHz before real work starts
            # (the PE runs at 1.2 GHz until it has been busy ~3.4us).
            warm = persist.tile([128, 512], F16)
            nc.vector.memset(warm[:], 0.0)
            with tc.tile_pool(name="pswm", bufs=1, space="PSUM") as pswm:
                wps = pswm.tile([128, 512], F32)
                # 9 x 427ns (cold) covers the ~3.4us HAM activity window
                # without delaying the first data-dependent matmul
                for i in range(9):
                    nc.tensor.matmul(
                        wps[:], warm[:, 0:128], warm[:],
                        start=(i == 0), stop=(i == 8),
                    )

            # K^T (roped) per head and V blocks, resident through attention
            kT_c = [kvres.tile([128, GH, QC], F16, tag=f"kT{c}", name=f"kT{c}")
                    for c in range(NQC)]
            # V with a ones column appended per head: feeds the ones-column
            # AV matmul that yields O and the softmax sum R
            v_c = [kvres.tile([128, 4, GH, HD + 1], F16,
                              tag=f"v{c}", name=f"v{c}") for c in range(NQC)]
            for c in range(NQC):
                nc.vector.memset(v_c[c][:, :, :, HD:], 1.0)

            def rope(dst, src_ps, tmp_pool, s0, n):
                """dst[:] = rope(src_ps) using cos/sin slices [s0:s0+n]."""
                tmp = tmp_pool.tile([128, SB], F16, tag="rope_tmp")
                nc.vector.tensor_mul(
                    tmp[0:64, :n], src_ps[64:128, :], sin_sb[0:64, s0:s0 + n]
                )
                nc.vector.tensor_mul(
                    tmp[64:128, :n], src_ps[0:64, :], sin_sb[64:128, s0:s0 + n]
                )
                nc.vector.tensor_mul(dst, src_ps[:, :], cos_sb[:, s0:s0 + n])
                nc.gpsimd.tensor_add(dst, dst, tmp[:, :n])

            # ---------------- stage 1: KV path ----------------
            with (
                tc.tile_pool(name="w1", bufs=1) as w1,
                tc.tile_pool(name="kvl", bufs=3) as kvlp,
                tc.tile_pool(name="tmp1", bufs=2) as tmp1,
                tc.tile_pool(name="ps1", bufs=2, space="PSUM") as ps1,
            ):
                wl_sb = w1.tile([128, KT, LATENT], F16)
                wuk_sb = w1.tile([128, LT, GD], F16)
                wuv_sb = w1.tile([128, LT, GD], F16)
                # first s-block of x goes out first (gates first matmul),
                # then weights in per-ko chunks
                xt0_sb = xtp.tile([128, KT, SB], F16, tag="xt", name="xt0")
                # interleave x-block-0 and Wl chunks: the first kvl matmul
                # needs xt0 chunk 0 + wl[ko=0..1] only, so lead with a
                # finer-grained pair of 2-ko chunks
                for kg2 in range(8):
                    nc.sync.dma_start(
                        xt0_sb[:, 2 * kg2:2 * kg2 + 2, :],
                        xT_t[:, 2 * kg2:2 * kg2 + 2, 0:SB],
                    )
                    for ko in range(2 * kg2, 2 * kg2 + 2):
                        nc.sync.dma_start(wl_sb[:, ko, :], wl_t[:, ko, :])
                # rope constants: the first s-block's slice is needed at
                # ~15us (first kT rope); the rest can trail the Wu loads
                nc.sync.dma_start(cos_sb[:, 0:SB], cosT[:, 0:SB])
                nc.sync.dma_start(sin_sb[:, 0:SB], sinT[:, 0:SB])
                nc.sync.dma_start(wuk_sb[:], wuk_t)
                nc.sync.dma_start(wuv_sb[:], wuv_t)
                nc.sync.dma_start(cos_sb[:, SB:], cosT[:, SB:])
                nc.sync.dma_start(sin_sb[:, SB:], sinT[:, SB:])
                nc.sync.dma_start(eye_sb[:], eyed[:, :])
                nc.sync.dma_start(masks_sb[:], masksd[:, :])

                for sb in range(NSB):
                    s0 = sb * SB
                    if sb == 0:
                        xt_sb = xt0_sb
                    else:
                        xt_sb = xtp.tile([128, KT, SB], F16, tag="xt")
                        for kg in range(4):  # 4-ko chunks
                            nc.sync.dma_start(
                                xt_sb[:, 4 * kg:4 * kg + 4, :],
                                xT_t[:, 4 * kg:4 * kg + 4, s0:s0 + SB],
                            )

                    # kv_latent^T [128, LT, SB] (fp16 for downstream matmuls)
                    kvl_sb = kvlp.tile([128, LT, SB], F16, tag="kvl")
                    for lo in range(LT):
                        ps = ps1.tile([128, SB], F32, tag="kvl_ps")
                        for ko in range(KT):
                            nc.tensor.matmul(
                                ps[:],
                                wl_sb[:, ko, lo * 128:(lo + 1) * 128],
                                xt_sb[:, ko, :],
                                start=(ko == 0),
                                stop=(ko == KT - 1),
                            )
                        # ACT is idle in stage 1; keep DVE free for rope
                        nc.scalar.copy(kvl_sb[:, lo, :], ps[:])

                    # K^T per head (+rope); SB == QC so block sb maps to
                    # chunk sb directly
                    for hh in range(GH):
                        ps = ps1.tile([128, SB], F32, tag="kT_ps")
                        for lo in range(LT):
                            nc.tensor.matmul(
                                ps[:],
                                wuk_sb[:, lo, hh * 128:(hh + 1) * 128],
                                kvl_sb[:, lo, :],
                                start=(lo == 0),
                                stop=(lo == LT - 1),
                            )
                        rope(kT_c[sb][:, hh, :], ps, tmp1, s0, SB)

                    # V in [s, d] layout: s-chunks of 128
                    for sc in range(SB // 128):
                        j = (s0 + sc * 128) // 128
                        ps = ps1.tile([128, GD], F32, tag="v_ps")
                        for lo in range(LT):
                            nc.tensor.matmul(
                                ps[:],
                                kvl_sb[:, lo, sc * 128:(sc + 1) * 128],
                                wuv_sb[:, lo, :],
                                start=(lo == 0),
                                stop=(lo == LT - 1),
                            )
                        nc.scalar.copy(
                            v_c[j // 4][:, j % 4, :, :HD],
                            ps.rearrange("p (h d) -> p h d", h=GH),
                        )

            if stage <= 1:
                nc.sync.dma_start(pout[0:128, 0:516], v_c[0][:, 0, :, :])
                nc.sync.dma_start(
                    pout[128:256, 0:512], kT_c[0][:, 0, 0:512]
                )
                cs.release()
                xtp.release()
                qres.release()
                return

            # ------- stage 2: Q chunks interleaved with attention -------
            with (
                tc.tile_pool(name="qtp", bufs=4) as qtp,
                tc.tile_pool(name="tmp2", bufs=2) as tmp2,
                tc.tile_pool(name="att", bufs=4) as attp,
                tc.tile_pool(name="attr", bufs=2) as attrp,
            ):
                pslg = tc.alloc_tile_pool(name="ps_lg", bufs=2, space="PSUM")
                psot = tc.alloc_tile_pool(name="ps_ot", bufs=1, space="PSUM")
                ps2 = tc.alloc_tile_pool(name="ps2", bufs=2, space="PSUM",
                                         side="right")
                wqp = tc.alloc_tile_pool(name="wqp", bufs=1, side="right")
                wq_sb = wqp.tile([128, KT, GD], F16)
                for ko in range(KT):
                    nc.sync.dma_start(wq_sb[:, ko, :], wq_t[:, ko, :])

                # normalized attention output per q-chunk, [q, s-sub, d]
                # layout; transposed on PE in the phase-3 tail
                o_c = [qres.tile([128, 4, GD], F16, tag=f"o{c}",
                                 name=f"o{c}") for c in range(NQC)]
                qT_c = [None] * NQC

                def q_chunk(c):
                    """Q projection + rope for q-chunk c (one 512 s-block)."""
                    qT_c[c] = qtp.tile([128, GH, QC], F16, tag="qTc",
                                       name=f"qT{c}")
                    s0 = c * SB
                    xt_sb = xtp.tile([128, KT, SB], F16, tag="xt", name="xt2")
                    for kg in range(4):
                        nc.sync.dma_start(
                            xt_sb[:, 4 * kg:4 * kg + 4, :],
                            xT_t[:, 4 * kg:4 * kg + 4, s0:s0 + SB],
                        )
                    for hh in range(GH):
                        ps = ps2.tile([128, SB], F32, tag="qT_ps")
                        for ko in range(KT):
                            nc.tensor.matmul(
                                ps[:],
                                wq_sb[:, ko, hh * 128:(hh + 1) * 128],
                                xt_sb[:, ko, :],
                                start=(ko == 0),
                                stop=(ko == KT - 1),
                            )
                        rope(qT_c[c][:, hh, :], ps, tmp2, s0, SB)

                def attn(qi, heads=range(GH)):
                    njb = 4 * qi + 4  # causal: k-blocks 0..4*qi+3
                    for hh in heads:
                        # one [128,129] accumulator per 128-wide q-sub:
                        # cols 0:128 = O (q rows, d cols), col 128 = R
                        o_qs = [psot.tile([128, HD + 1], F32, tag=f"oq{s}",
                                          bufs=1, name=f"oq{s}")
                                for s in range(4)]

                        def qk(j):
                            """QK^T + exp + causal mask for k-block j."""
                            t = j - 4 * qi
                            # causal: q-cols < 128*t fully masked (fp16
                            # matmuls have no minimum-width penalty)
                            qoff = 0 if t < 1 else 128 * t
                            nw = QC - qoff
                            lg = pslg.tile([128, QC], F32, tag="lg")
                            nc.tensor.matmul(
                                lg[:, :nw],
                                kT_c[j // 4][:, hh,
                                             (j % 4) * 128:(j % 4 + 1) * 128],
                                qT_c[qi][:, hh, qoff:],
                                start=True,
                                stop=True,
                            )
                            e_sb = attp.tile([128, QC], F16, tag="e")
                            nc.scalar.activation(
                                e_sb[:, :nw],
                                lg[:, :nw],
                                mybir.ActivationFunctionType.Exp,
                                scale=SCALE,
                            )
                            if t >= 0:
                                # only the first 128 q-cols of a diagonal
                                # block are ever masked
                                nc.vector.tensor_mul(
                                    e_sb[:, :128], e_sb[:, :128],
                                    masks_sb[:, :],
                                )
                            return e_sb

                        def av(j, e_sb):
                            t = j - 4 * qi
                            qoff = 0 if t < 1 else 128 * t
                            for s in range(max(t, 0), 4):
                                nc.tensor.matmul(
                                    o_qs[s][:, :],
                                    e_sb[:, s * 128 - qoff:
                                         (s + 1) * 128 - qoff],
                                    v_c[j // 4][:, j % 4, hh, :],
                                    start=(j == 0),
                                    stop=(j == 4 * qi + s),
                                )

                        # software-pipeline: QK(j+1) is emitted before
                        # AV(j) so the PE streams QKs during exp(j)
                        e_prev = qk(0)
                        for j in range(1, njb):
                            e_cur = qk(j)
                            av(j - 1, e_prev)
                            e_prev = e_cur
                        av(njb - 1, e_prev)
                        # normalize rows: O[q, :] / R[q] (per-partition scalar)
                        for s in range(4):
                            rec = attp.tile([128, 1], F32, tag="rec")
                            nc.vector.reciprocal(rec[:], o_qs[s][:, HD:])
                            nc.vector.tensor_scalar_mul(
                                o_c[qi][:, s, hh * HD:(hh + 1) * HD],
                                o_qs[s][:, :HD],
                                rec[:],
                            )

                # big-to-small: overlap attn(3)'s exp-heavy stream with the
                # remaining (PE-only) Q projections
                q_chunk(3)
                attn(3, heads=[0])
                q_chunk(2)
                attn(3, heads=[1])
                q_chunk(1)
                attn(3, heads=[2, 3])
                q_chunk(0)
                wqp.release()
                cs.release()
                xtp.release()
                ps2.release()

                if stage <= 2:
                    nc.sync.dma_start(
                        pout[256:384, 0:512], qT_c[0][:, 0, :]
                    )
                    return

                # ---- phase 3 fused: project finished q-chunks while the
                # remaining attention chunks run (fills ACT-bound PE gaps)
                with (
                    tc.tile_pool(name="w3", bufs=1) as w3,
                    tc.tile_pool(name="otr", bufs=2) as otrp,
                    tc.tile_pool(name="osb", bufs=4) as osbp,
                    tc.tile_pool(name="ps3", bufs=1, space="PSUM",
                                 side="right") as ps3,
                ):
                    wp_sb = w3.tile([128, LT, DOUT], F16)
                    for dt_ in range(LT):
                        nc.sync.dma_start(wp_sb[:, dt_, :], wp_t[:, dt_, :])

                    def ph3(qi, sc, po_pool=None, tr_pool=None):
                        q0 = qi * QC
                        tr = (tr_pool or ps3).tile([128, 512], F16, tag="tr",
                                                   bufs=1 if tr_pool is None
                                                   else None, name="tr")
                        for dt_ in range(LT):
                            nc.tensor.transpose(
                                tr[:, dt_ * HD:(dt_ + 1) * HD],
                                o_c[qi][:, sc, dt_ * HD:(dt_ + 1) * HD],
                                eye_sb[:],
                            )
                        oT_sb = otrp.tile([128, 512], F16, tag="ot")
                        nc.vector.tensor_copy(oT_sb[:], tr[:])
                        for ec in range(DOUT // 512):
                            if po_pool is None:
                                po = ps3.tile([128, 512], F32, tag="po",
                                              bufs=1)
                            else:
                                po = po_pool.tile([128, 512], F32, tag="po2",
                                                  name="po2")
                            for dt_ in range(LT):
                                nc.tensor.matmul(
                                    po[:],
                                    oT_sb[:, dt_ * HD:(dt_ + 1) * HD],
                                    wp_sb[:, dt_, ec * 512:(ec + 1) * 512],
                                    start=(dt_ == 0),
                                    stop=(dt_ == LT - 1),
                                )
                            out_sb = osbp.tile([128, 512], F16, tag="out")
                            # ACT runs the exp stream in this phase —
                            # split the psum drain between ACT and DVE
                            if ec % 2 == 0:
                                nc.vector.tensor_copy(out_sb[:], po[:])
                            else:
                                nc.scalar.copy(out_sb[:], po[:])
                            nc.sync.dma_start(
                                pout[q0 + sc * 128:q0 + (sc + 1) * 128,
                                     ec * 512:(ec + 1) * 512],
                                out_sb[:],
                            )

                    for x in range(4):
                        attn(2, heads=[x])
                        ph3(3, x)
                    for x in range(4):
                        attn(1, heads=[x])
                        ph3(2, x)
                    for x in range(4):
                        attn(0, heads=[x])
                        ph3(1, x)
                    # attention done: hand its PSUM banks to the tail
                    psot.release()
                    pslg.release()
                    ps4 = tc.alloc_tile_pool(name="ps4", bufs=4,
                                             space="PSUM", side="right")
                    ps5 = tc.alloc_tile_pool(name="ps5", bufs=2,
                                             space="PSUM", side="right")
                    for x in range(4):
                        ph3(0, x, po_pool=ps4, tr_pool=ps5)
                    ps5.release()
                    ps4.release()
            qres.release()


_CACHE: dict = {}


def _get_nc():
    if "nc" not in _CACHE:
        _CACHE["nc"] = build_nc()
    return _CACHE["nc"]


def _host_inputs(x, position_embeddings, Wq, Wl, Wu, Wp):
    x = np.asarray(x, dtype=np.float32)
    pe = np.asarray(position_embeddings, dtype=np.float32)[:S]
    Wq = np.asarray(Wq, dtype=np.float16)
    Wl = np.asarray(Wl, dtype=np.float16)
    Wu = np.asarray(Wu, dtype=np.float16)
    Wp = np.asarray(Wp, dtype=np.float16)

    cos = np.ascontiguousarray(np.cos(pe).T)          # [HD, S]
    sinF = np.ascontiguousarray(np.sin(pe).T)         # [HD, S]
    sinF[: HD // 2] *= -1.0                           # fold rotate-half sign

    # mask for the first 128 q-cols of a diagonal block: M[k, c] = c >= k
    k = np.arange(128)[:, None]
    c = np.arange(128)[None, :]
    masks = np.ascontiguousarray((c >= k).astype(np.float16))

    xTs = [np.ascontiguousarray(x[b].T).astype(np.float16) for b in range(B)]

    in_maps = []
    for cc in range(NCORES):
        b, g = divmod(cc, GROUPS)
        in_maps.append({
            "xT": xTs[b],
            "wq": np.ascontiguousarray(Wq[:, g * GD:(g + 1) * GD]),
            "wl": Wl,
            "wuk": np.ascontiguousarray(Wu[:, g * GD:(g + 1) * GD]),
            "wuv": np.ascontiguousarray(
                Wu[:, DOUT + g * GD:DOUT + (g + 1) * GD]
            ),
            "wp": np.ascontiguousarray(Wp[g * GD:(g + 1) * GD, :]),
            "cosT": cos,
            "sinT": sinF,
            "masks": masks,
            "eye": np.eye(128, dtype=np.float16),
        })
    return in_maps


def run(x, position_embeddings, Wq, Wl, Wu, Wp, trace=False):
    """Run on 8 cores; returns (output, BassKernelResults)."""
    nc = _get_nc()
    in_maps = _host_inputs(x, position_embeddings, Wq, Wl, Wu, Wp)
    res = bass_utils.run_bass_kernel_spmd(
        nc, in_maps, core_ids=list(range(NCORES)), trace=trace,
        trace_cores=list(range(NCORES)) if trace else None,
    )
    parts = [r["pout"] for r in res.results]
    out = np.empty((B, S, DOUT), dtype=np.float32)
    for b in range(B):
        out[b] = np.sum(
            np.stack([p.astype(np.float32)
                      for p in parts[b * GROUPS:(b + 1) * GROUPS]]),
            axis=0,
        )
    return out, res


def kernel(x, position_embeddings, Wq, Wl, Wu, Wp):
    out, _ = run(x, position_embeddings, Wq, Wl, Wu, Wp, trace=False)
    return out
